# revision 19
# baseline (speedup 1.0000x reference)
"""GQA attention layer (dense transformer block) on 8 TRN2 NeuronCores.

Tensor-parallel sharding over heads: each core owns 4 q-heads + 1 kv-head
(wq/wk/wv column shards, wo row shard), computes a partial output
[2048, 2048], and the host sums the 8 partials (the row-parallel
all-reduce) to produce the full output.

Per-core dataflow (activations transposed, [feature, seq]; SBUF bf16,
PSUM fp32):
  x preloaded to SBUF in 4 slab DMAs (stage A never waits on HBM)
  qT/kvT projections kc-outer so one weight load feeds 2 matmuls
  RoPE via a [128,128] +-1 rotation matmul + DVE combine with cos/sin
  scores for a head PAIR issued back-to-back: the two 64-contraction
    matmuls land in distinct PE row-groups (auto tile_position) and overlap
  E = exp(ST/8) for both heads in one ACT op (psum->sbuf bf16)
  [oT_h; rowsum] = [v|1].T @ E    (PE accumulate over key chunks)
  oT_h *= 1/rowsum  (DVE reciprocal + gpsimd partition-broadcast)
  out_partial = oT.T @ wo_c  (fp32 to HBM)
"""
import sys

sys.path.insert(0, "/opt/trn_rl_repo")

import numpy as np
import ml_dtypes
import concourse.bass as bass
import concourse.mybir as mybir
import concourse.tile as tile
from concourse import bacc
from concourse.bass_utils import run_bass_kernel_spmd

F32 = mybir.dt.float32
BF = mybir.dt.bfloat16
AF = mybir.ActivationFunctionType
NPBF = ml_dtypes.bfloat16

S = 2048          # sequence length
D = 2048          # model dim
HD = 64           # head dim
HLOC = 4          # q heads per core
NCORES = 8
QW = HLOC * HD    # 256, local q width
KC = S // 128     # 16 key chunks
NS = 4            # x / q-span slices of 512
ROPE_BASE = 10000.0
SCALE = 0.125     # 1/sqrt(HD), applied inside exp


def _build_program():
    nc = bacc.Bacc(None, target_bir_lowering=False)

    xt = nc.dram_tensor("xt", [D, S], BF, kind="ExternalInput")
    wq_d = nc.dram_tensor("wq_s", [D, QW], BF, kind="ExternalInput")
    wkv_d = nc.dram_tensor("wkv_s", [D, 128], BF, kind="ExternalInput")
    wo_d = nc.dram_tensor("wo_s", [QW, D], BF, kind="ExternalInput")
    cos_d = nc.dram_tensor("cos2", [128, S], BF, kind="ExternalInput")
    sin_d = nc.dram_tensor("sin2", [128, S], BF, kind="ExternalInput")
    rotq_d = nc.dram_tensor("rot_q", [128, 128], BF, kind="ExternalInput")
    rotk_d = nc.dram_tensor("rot_k", [128, 64], BF, kind="ExternalInput")
    id64_d = nc.dram_tensor("id64", [128, 64], BF, kind="ExternalInput")
    ones_d = nc.dram_tensor("ones_col", [128, KC], BF, kind="ExternalInput")
    out_d = nc.dram_tensor("out", [S, D], F32, kind="ExternalOutput")

    with tile.TileContext(nc) as tc:
        with (
            tc.tile_pool(name="consts", bufs=1) as consts,
            tc.tile_pool(name="big", bufs=1) as big,
        ):
            # x slabs on the HW DGE (sync); bulk weights/constants on the
            # SW DGE (gpsimd) so the two streams run on different rings.
            xall = big.tile([128, KC, S], BF)
            xre = xt.ap().rearrange("(c p) s -> p c s", p=128)
            for lo, hi in ((0, 2), (2, 6), (6, 11), (11, 16)):
                nc.sync.dma_start(xall[:, lo:hi, :], xre[:, lo:hi, :])

            wq_sb = consts.tile([128, KC, QW], BF)
            nc.gpsimd.dma_start(wq_sb[:], wq_d.ap().rearrange("(c p) m -> p c m", p=128))
            wkv_sb = consts.tile([128, KC, 128], BF)
            nc.gpsimd.dma_start(wkv_sb[:], wkv_d.ap().rearrange("(c p) m -> p c m", p=128))
            rotq_sb = consts.tile([128, 128], BF)
            nc.gpsimd.dma_start(rotq_sb[:], rotq_d[:, :])
            rotk_sb = consts.tile([128, 64], BF)
            nc.gpsimd.dma_start(rotk_sb[:], rotk_d[:, :])
            id64_sb = consts.tile([128, 64], BF)
            nc.gpsimd.dma_start(id64_sb[:], id64_d[:, :])
            cos_sb = consts.tile([128, S], BF)
            nc.gpsimd.dma_start(cos_sb[:], cos_d[:, :])
            sin_sb = consts.tile([128, S], BF)
            nc.gpsimd.dma_start(sin_sb[:], sin_d[:, :])
            wo_sb = consts.tile([128, 2, D], BF)
            nc.gpsimd.dma_start(wo_sb[:], wo_d.ap().rearrange("(b p) e -> p b e", p=128))

            # persistent activations
            qTr = [big.tile([128, S], BF, name=f"qTr{j}", tag=f"qTr{j}") for j in range(2)]
            kTr = big.tile([128, S], BF)  # k-rope duplicated in both halves
            kvT = big.tile([128, S], BF)
            v_aug = big.tile([128, KC, 65], BF)
            nc.gpsimd.dma_start(v_aug[:, :, 64:65], ones_d.ap().rearrange("p (c o) -> p c o", o=1))
            oT = [big.tile([128, S], BF, name=f"oT{j}", tag=f"oT{j}") for j in range(2)]

            # ---------------- stage A: projections + rope + v transpose
            # n-outer with double-buffered accumulators: rope of slice n
            # overlaps the projections of slice n+1, so the PE stream stays
            # dense (x is preloaded, so no DMA waits either).
            with (
                tc.tile_pool(name="psA", bufs=1, space="PSUM") as psA,
                tc.tile_pool(name="tmpA", bufs=3) as tmpA,
            ):
                for n in range(NS):
                    nsl = bass.ts(n, 512)
                    q0_ps = psA.tile([128, 512], F32, tag="q0", bufs=2)
                    q1_ps = psA.tile([128, 512], F32, tag="q1", bufs=2)
                    kv_ps = psA.tile([128, 512], F32, tag="kv", bufs=2)
                    for kc in range(KC):
                        st_ = kc == 0
                        sp_ = kc == KC - 1
                        xsl = xall[:, kc, nsl]
                        nc.tensor.matmul(q0_ps[:], wq_sb[:, kc, 0:128], xsl, start=st_, stop=sp_)
                        nc.tensor.matmul(q1_ps[:], wq_sb[:, kc, 128:256], xsl, start=st_, stop=sp_)
                        nc.tensor.matmul(kv_ps[:], wkv_sb[:, kc, :], xsl, start=st_, stop=sp_)

                    # rope for the two q tiles
                    for jb, ps in ((0, q0_ps), (1, q1_ps)):
                        q_sb = tmpA.tile([128, 512], BF, tag=f"q{jb}sb")
                        nc.scalar.copy(q_sb[:], ps[:])
                        rot_ps = psA.tile([128, 512], F32, tag="rot", bufs=1)
                        nc.tensor.matmul(rot_ps[:], rotq_sb[:], q_sb[:], start=True, stop=True)
                        t_cos = tmpA.tile([128, 512], BF, tag="tc", bufs=2)
                        nc.vector.tensor_mul(t_cos[:], q_sb[:], cos_sb[:, nsl])
                        t_sin = tmpA.tile([128, 512], BF, tag="tsn", bufs=2)
                        nc.vector.tensor_mul(t_sin[:], rot_ps[:], sin_sb[:, nsl])
                        nc.vector.tensor_add(qTr[jb][:, nsl], t_cos[:], t_sin[:])

                    # kv: copy, k-rope, v transpose
                    nc.scalar.copy(kvT[:, nsl], kv_ps[:])
                    rk_ps = psA.tile([128, 512], F32, tag="rot", bufs=1)
                    nc.tensor.matmul(rk_ps[0:64, :], rotk_sb[:], kvT[:, nsl], start=True, stop=True)
                    tk_cos = tmpA.tile([128, 512], BF, tag="tc", bufs=2)
                    nc.vector.tensor_mul(tk_cos[0:64, :], kvT[0:64, nsl], cos_sb[0:64, nsl])
                    tk_sin = tmpA.tile([128, 512], BF, tag="tsn", bufs=2)
                    nc.vector.tensor_mul(tk_sin[0:64, :], rk_ps[0:64, :], sin_sb[0:64, nsl])
                    nc.vector.tensor_add(kTr[0:64, nsl], tk_cos[0:64, :], tk_sin[0:64, :])
                    nc.vector.tensor_add(kTr[64:128, nsl], tk_cos[0:64, :], tk_sin[0:64, :])

                    for j in range(4):
                        ck = 4 * n + j
                        vt_ps = psA.tile([128, 64], BF, tag="vt", bufs=1)
                        nc.tensor.transpose(
                            vt_ps[:],
                            kvT[64:128, ck * 128:(ck + 1) * 128],
                            id64_sb[64:128, :],
                        )
                        nc.scalar.copy(v_aug[:, ck, 0:64], vt_ps[:])

            # ---------------- stage B: attention, stage C: output projection
            with (
                tc.tile_pool(name="psB", bufs=1, space="PSUM") as psB,
                tc.tile_pool(name="psC", bufs=1, space="PSUM") as psC,
                tc.tile_pool(name="tmpB", bufs=2) as tmpB,
                tc.tile_pool(name="outp", bufs=3) as outp,
            ):
                def stage_c(cq, s_lo, s_hi):
                    for st4 in range(s_lo, s_hi):
                        srow = cq * 4 + st4
                        for nn in range(NS):
                            o_ps = psC.tile([128, 512], F32, tag="oc", bufs=2)
                            nc.tensor.matmul(
                                o_ps[:], oT[0][:, srow * 128:(srow + 1) * 128],
                                wo_sb[:, 0, bass.ts(nn, 512)], start=True, stop=False,
                            )
                            nc.tensor.matmul(
                                o_ps[:], oT[1][:, srow * 128:(srow + 1) * 128],
                                wo_sb[:, 1, bass.ts(nn, 512)], start=False, stop=True,
                            )
                            ob = outp.tile([128, 512], F32, tag="ob")
                            nc.vector.tensor_copy(ob[:], o_ps[:])
                            nc.sync.dma_start(
                                out_d[srow * 128:(srow + 1) * 128, bass.ts(nn, 512)], ob[:]
                            )

                for qq in range(NS):
                    qsl = bass.ts(qq, 512)
                    for jb in range(2):  # head pair: heads 2*jb + {0,1}
                        ot_a = psB.tile([65, 512], F32, tag="ota", bufs=1)
                        ot_b = psB.tile([65, 512], F32, tag="otb", bufs=1)
                        prev = None

                        def av(pair):
                            kc, e = pair
                            st_ = kc == 0
                            sp_ = kc == KC - 1
                            nc.tensor.matmul(ot_a[:], v_aug[:, kc, :], e[:, 0, :], start=st_, stop=sp_)
                            nc.tensor.matmul(ot_b[:], v_aug[:, kc, :], e[:, 1, :], start=st_, stop=sp_)

                        for kc in range(KC):
                            st_ps = psB.tile([128, 2, 512], F32, tag="st", bufs=2)
                            ksl = slice(kc * 128, (kc + 1) * 128)
                            # the two 64-contraction score MMs sit in distinct
                            # PE row-groups (base partitions 0 / 64) -> overlap
                            nc.tensor.matmul(st_ps[:, 0, :], kTr[0:64, ksl],
                                             qTr[jb][0:64, qsl], start=True, stop=True)
                            nc.tensor.matmul(st_ps[:, 1, :], kTr[64:128, ksl],
                                             qTr[jb][64:128, qsl], start=True, stop=True)
                            if prev is not None:
                                av(prev)
                            e_sb = tmpB.tile([128, 2, 512], BF, tag="e")
                            nc.scalar.activation(e_sb[:], st_ps[:], AF.Exp, scale=SCALE)
                            prev = (kc, e_sb)
                        av(prev)

                        for rr, otp in ((0, ot_a), (1, ot_b)):
                            recip = tmpB.tile([1, 512], F32, tag="recip")
                            nc.vector.reciprocal(recip[:], otp[64:65, :])
                            bcast = tmpB.tile([64, 512], F32, tag="bcast")
                            nc.gpsimd.partition_broadcast(bcast[:], recip[:])
                            nc.vector.tensor_mul(
                                oT[jb][rr * 64:rr * 64 + 64, qsl], otp[0:64, :], bcast[:]
                            )

                        # stage C of the previous q span, 2 srows per head
                        # pair: independent PE work emitted right where the
                        # next pair's first AV waits on the ot slots, hiding
                        # the softmax-chain latency and keeping HAM warm
                        if qq > 0:
                            stage_c(qq - 1, 2 * jb, 2 * jb + 2)
                    if qq == NS - 1:
                        stage_c(qq, 0, 4)
    nc.compile()
    return nc


_NC_CACHE = None


def _get_program():
    global _NC_CACHE
    if _NC_CACHE is None:
        _NC_CACHE = _build_program()
    return _NC_CACHE


def _host_constants():
    inv_freq = 1.0 / (ROPE_BASE ** (np.arange(0, HD, 2, dtype=np.float32) / HD))
    t = np.arange(S, dtype=np.float32)
    freqs = np.outer(t, inv_freq)
    emb = np.concatenate([freqs, freqs], -1)          # [s, 64]
    cosT = np.cos(emb).T.astype(np.float32)           # [64, s]
    sinT = np.sin(emb).T.astype(np.float32)
    cos2 = np.ascontiguousarray(np.concatenate([cosT, cosT], 0))  # [128, s]
    sin2 = np.ascontiguousarray(np.concatenate([sinT, sinT], 0))

    R = np.zeros((HD, HD), np.float32)
    for i in range(32):
        R[i, i + 32] = -1.0
        R[i + 32, i] = 1.0
    RT = R.T
    rot_q = np.zeros((128, 128), np.float32)
    rot_q[0:64, 0:64] = RT
    rot_q[64:128, 64:128] = RT
    rot_k = np.zeros((128, 64), np.float32)
    rot_k[0:64, 0:64] = RT
    id64 = np.zeros((128, 64), np.float32)
    id64[64:128, :] = np.eye(64, dtype=np.float32)
    ones_col = np.ones((128, KC), np.float32)
    return cos2, sin2, rot_q, rot_k, id64, ones_col


def _bf(a):
    return np.ascontiguousarray(np.asarray(a, dtype=np.float32)).astype(NPBF)


def _in_maps(x, wq, wk, wv, wo):
    xT = _bf(x.reshape(S, D).T)
    cos2, sin2, rot_q, rot_k, id64, ones_col = _host_constants()
    cos2, sin2, rot_q, rot_k, id64, ones_col = (
        _bf(cos2), _bf(sin2), _bf(rot_q), _bf(rot_k), _bf(id64), _bf(ones_col)
    )
    maps = []
    for c in range(NCORES):
        wq_c = _bf(wq[:, c * QW:(c + 1) * QW])
        wkv_c = _bf(
            np.concatenate([wk[:, c * HD:(c + 1) * HD], wv[:, c * HD:(c + 1) * HD]], 1)
        )
        wo_c = _bf(wo[c * QW:(c + 1) * QW, :])
        maps.append({
            "xt": xT, "wq_s": wq_c, "wkv_s": wkv_c, "wo_s": wo_c,
            "cos2": cos2, "sin2": sin2, "rot_q": rot_q, "rot_k": rot_k,
            "id64": id64, "ones_col": ones_col,
        })
    return maps


def _run(in_maps, trace=False):
    nc = _get_program()
    return run_bass_kernel_spmd(nc, in_maps, core_ids=list(range(NCORES)), trace=trace)


def kernel(x, wq, wk, wv, wo):
    x, wq, wk, wv, wo = (np.asarray(a, dtype=np.float32) for a in (x, wq, wk, wv, wo))
    res = _run(_in_maps(x, wq, wk, wv, wo), trace=False)
    acc = res.results[0]["out"].astype(np.float64)
    for c in range(1, NCORES):
        acc += res.results[c]["out"]
    return acc.astype(np.float32).reshape(1, S, D)


def run_traced(x, wq, wk, wv, wo):
    """Like kernel() but with NTFF profiling; returns (out, BassKernelResults)."""
    x, wq, wk, wv, wo = (np.asarray(a, dtype=np.float32) for a in (x, wq, wk, wv, wo))
    res = _run(_in_maps(x, wq, wk, wv, wo), trace=True)
    acc = res.results[0]["out"].astype(np.float64)
    for c in range(1, NCORES):
        acc += res.results[c]["out"]
    return acc.astype(np.float32).reshape(1, S, D), res


# revision 21
# speedup vs baseline: 1.1103x; 1.1103x over previous
"""GQA attention layer (dense transformer block) on 8 TRN2 NeuronCores.

Tensor-parallel sharding over heads: each core owns 4 q-heads + 1 kv-head
(wq/wk/wv column shards, wo row shard), computes a partial output
[2048, 2048], and the host sums the 8 partials (the row-parallel
all-reduce) to produce the full output.

Per-core dataflow (activations transposed, [feature, seq]; SBUF bf16,
PSUM fp32):
  x preloaded to SBUF in 4 slab DMAs (stage A never waits on HBM)
  qT/kvT projections kc-outer so one weight load feeds 2 matmuls
  RoPE via a [128,128] +-1 rotation matmul + DVE combine with cos/sin
  scores for a head PAIR issued back-to-back: the two 64-contraction
    matmuls land in distinct PE row-groups (auto tile_position) and overlap
  E = exp(ST/8) for both heads in one ACT op (psum->sbuf bf16)
  [oT_h; rowsum] = [v|1].T @ E    (PE accumulate over key chunks)
  oT_h *= 1/rowsum  (DVE reciprocal + gpsimd partition-broadcast)
  out_partial = oT.T @ wo_c  (fp32 to HBM)
"""
import sys

sys.path.insert(0, "/opt/trn_rl_repo")

import numpy as np
import ml_dtypes
import concourse.bass as bass
import concourse.mybir as mybir
import concourse.tile as tile
from concourse import bacc
from concourse.bass_utils import run_bass_kernel_spmd

F32 = mybir.dt.float32
BF = mybir.dt.bfloat16
AF = mybir.ActivationFunctionType
NPBF = ml_dtypes.bfloat16

S = 2048          # sequence length
D = 2048          # model dim
HD = 64           # head dim
HLOC = 4          # q heads per core
NCORES = 8
QW = HLOC * HD    # 256, local q width
KC = S // 128     # 16 key chunks
NS = 4            # x / q-span slices of 512
ROPE_BASE = 10000.0
SCALE = 0.125     # 1/sqrt(HD), applied inside exp


def _build_program():
    nc = bacc.Bacc(None, target_bir_lowering=False)

    xt = nc.dram_tensor("xt", [D, S], BF, kind="ExternalInput")
    wq_d = nc.dram_tensor("wq_s", [D, QW], BF, kind="ExternalInput")
    wkv_d = nc.dram_tensor("wkv_s", [D, 128], BF, kind="ExternalInput")
    wo_d = nc.dram_tensor("wo_s", [QW, D], BF, kind="ExternalInput")
    cos_d = nc.dram_tensor("cos2", [128, S], BF, kind="ExternalInput")
    sin_d = nc.dram_tensor("sin2", [128, S], BF, kind="ExternalInput")
    rotq_d = nc.dram_tensor("rot_q", [128, 128], BF, kind="ExternalInput")
    rotk_d = nc.dram_tensor("rot_k", [128, 64], BF, kind="ExternalInput")
    id64_d = nc.dram_tensor("id64", [128, 64], BF, kind="ExternalInput")
    ones_d = nc.dram_tensor("ones_col", [128, KC], BF, kind="ExternalInput")
    out_d = nc.dram_tensor("out", [S, D], F32, kind="ExternalOutput")

    with tile.TileContext(nc) as tc:
        with (
            tc.tile_pool(name="consts", bufs=1) as consts,
            tc.tile_pool(name="big", bufs=1) as big,
        ):
            # Ring split tuned for the startup head: the first matmuls need
            # wq + wkv + x chunk 0, so wq leads the HW-DGE (sync) ring ahead
            # of the x slabs, wkv leads the SW-DGE (gpsimd) ring, and the
            # later-used constants (cos/sin/rot for rope, wo for stage C)
            # trail in first-use order.
            wq_sb = consts.tile([128, KC, QW], BF)
            nc.sync.dma_start(wq_sb[:], wq_d.ap().rearrange("(c p) m -> p c m", p=128))
            xall = big.tile([128, KC, S], BF)
            xre = xt.ap().rearrange("(c p) s -> p c s", p=128)
            for lo, hi in ((0, 2), (2, 6), (6, 11), (11, 16)):
                nc.sync.dma_start(xall[:, lo:hi, :], xre[:, lo:hi, :])

            wkv_sb = consts.tile([128, KC, 128], BF)
            nc.gpsimd.dma_start(wkv_sb[:], wkv_d.ap().rearrange("(c p) m -> p c m", p=128))
            rotq_sb = consts.tile([128, 128], BF)
            nc.gpsimd.dma_start(rotq_sb[:], rotq_d[:, :])
            cos_sb = consts.tile([128, S], BF)
            nc.gpsimd.dma_start(cos_sb[:], cos_d[:, :])
            sin_sb = consts.tile([128, S], BF)
            nc.gpsimd.dma_start(sin_sb[:], sin_d[:, :])
            rotk_sb = consts.tile([128, 64], BF)
            nc.gpsimd.dma_start(rotk_sb[:], rotk_d[:, :])
            id64_sb = consts.tile([128, 64], BF)
            nc.gpsimd.dma_start(id64_sb[:], id64_d[:, :])
            wo_sb = consts.tile([128, 2, D], BF)
            nc.gpsimd.dma_start(wo_sb[:], wo_d.ap().rearrange("(b p) e -> p b e", p=128))

            # persistent activations
            qTr = [big.tile([128, S], BF, name=f"qTr{j}", tag=f"qTr{j}") for j in range(2)]
            kTr = big.tile([128, S], BF)  # k-rope duplicated in both halves
            kvT = big.tile([128, S], BF)
            v_aug = big.tile([128, KC, 65], BF)
            nc.gpsimd.dma_start(v_aug[:, :, 64:65], ones_d.ap().rearrange("p (c o) -> p c o", o=1))
            oT = [big.tile([128, S], BF, name=f"oT{j}", tag=f"oT{j}") for j in range(2)]

            # ---------------- stage A: projections + rope + v transpose
            # n-outer with double-buffered accumulators: rope of slice n
            # overlaps the projections of slice n+1, so the PE stream stays
            # dense (x is preloaded, so no DMA waits either).
            with (
                tc.tile_pool(name="psA", bufs=1, space="PSUM") as psA,
                tc.tile_pool(name="tmpA", bufs=3) as tmpA,
            ):
                for n in range(NS):
                    nsl = bass.ts(n, 512)
                    q0_ps = psA.tile([128, 512], F32, tag="q0", bufs=2)
                    q1_ps = psA.tile([128, 512], F32, tag="q1", bufs=2)
                    kv_ps = psA.tile([128, 512], F32, tag="kv", bufs=2)
                    for kc in range(KC):
                        st_ = kc == 0
                        sp_ = kc == KC - 1
                        xsl = xall[:, kc, nsl]
                        nc.tensor.matmul(q0_ps[:], wq_sb[:, kc, 0:128], xsl, start=st_, stop=sp_)
                        nc.tensor.matmul(q1_ps[:], wq_sb[:, kc, 128:256], xsl, start=st_, stop=sp_)
                        nc.tensor.matmul(kv_ps[:], wkv_sb[:, kc, :], xsl, start=st_, stop=sp_)

                    # rope for the two q tiles
                    for jb, ps in ((0, q0_ps), (1, q1_ps)):
                        q_sb = tmpA.tile([128, 512], BF, tag=f"q{jb}sb")
                        nc.scalar.copy(q_sb[:], ps[:])
                        rot_ps = psA.tile([128, 512], F32, tag="rot", bufs=1)
                        nc.tensor.matmul(rot_ps[:], rotq_sb[:], q_sb[:], start=True, stop=True)
                        t_cos = tmpA.tile([128, 512], BF, tag="tc", bufs=2)
                        nc.vector.tensor_mul(t_cos[:], q_sb[:], cos_sb[:, nsl])
                        t_sin = tmpA.tile([128, 512], BF, tag="tsn", bufs=2)
                        nc.vector.tensor_mul(t_sin[:], rot_ps[:], sin_sb[:, nsl])
                        nc.vector.tensor_add(qTr[jb][:, nsl], t_cos[:], t_sin[:])

                    # kv: copy, k-rope, v transpose
                    nc.scalar.copy(kvT[:, nsl], kv_ps[:])
                    rk_ps = psA.tile([128, 512], F32, tag="rot", bufs=1)
                    nc.tensor.matmul(rk_ps[0:64, :], rotk_sb[:], kvT[:, nsl], start=True, stop=True)
                    tk_cos = tmpA.tile([128, 512], BF, tag="tc", bufs=2)
                    nc.vector.tensor_mul(tk_cos[0:64, :], kvT[0:64, nsl], cos_sb[0:64, nsl])
                    tk_sin = tmpA.tile([128, 512], BF, tag="tsn", bufs=2)
                    nc.vector.tensor_mul(tk_sin[0:64, :], rk_ps[0:64, :], sin_sb[0:64, nsl])
                    nc.vector.tensor_add(kTr[0:64, nsl], tk_cos[0:64, :], tk_sin[0:64, :])
                    nc.vector.tensor_add(kTr[64:128, nsl], tk_cos[0:64, :], tk_sin[0:64, :])

                    for j in range(4):
                        ck = 4 * n + j
                        vt_ps = psA.tile([128, 64], BF, tag="vt", bufs=1)
                        nc.tensor.transpose(
                            vt_ps[:],
                            kvT[64:128, ck * 128:(ck + 1) * 128],
                            id64_sb[64:128, :],
                        )
                        nc.scalar.copy(v_aug[:, ck, 0:64], vt_ps[:])

            # ---------------- stage B: attention, stage C: output projection
            with (
                tc.tile_pool(name="psB", bufs=1, space="PSUM") as psB,
                tc.tile_pool(name="psC", bufs=1, space="PSUM") as psC,
                tc.tile_pool(name="tmpB", bufs=2) as tmpB,
                tc.tile_pool(name="outp", bufs=3) as outp,
            ):
                def stage_c(cq, s_lo, s_hi):
                    for st4 in range(s_lo, s_hi):
                        srow = cq * 4 + st4
                        for nn in range(NS):
                            o_ps = psC.tile([128, 512], F32, tag="oc", bufs=2)
                            nc.tensor.matmul(
                                o_ps[:], oT[0][:, srow * 128:(srow + 1) * 128],
                                wo_sb[:, 0, bass.ts(nn, 512)], start=True, stop=False,
                            )
                            nc.tensor.matmul(
                                o_ps[:], oT[1][:, srow * 128:(srow + 1) * 128],
                                wo_sb[:, 1, bass.ts(nn, 512)], start=False, stop=True,
                            )
                            ob = outp.tile([128, 512], F32, tag="ob")
                            nc.vector.tensor_copy(ob[:], o_ps[:])
                            nc.sync.dma_start(
                                out_d[srow * 128:(srow + 1) * 128, bass.ts(nn, 512)], ob[:]
                            )

                ob0 = big.tile([128, 16, 512], F32, name="ob0", tag="ob0")

                for qq in range(NS):
                    qsl = bass.ts(qq, 512)
                    for h in range(HLOC):
                        jb, rr = divmod(h, 2)
                        q_rhs = qTr[jb][rr * 64:rr * 64 + 64, qsl]
                        ot_ps = psB.tile([65, 512], F32, tag="ot", bufs=2)
                        prev = None

                        def av(pair):
                            g, e = pair
                            for j in range(2):
                                kc = 2 * g + j
                                nc.tensor.matmul(
                                    ot_ps[:], v_aug[:, kc, :], e[:, j, :],
                                    start=(kc == 0), stop=(kc == KC - 1),
                                )

                        for g in range(KC // 2):
                            st_ps = psB.tile([128, 2, 512], F32, tag="st", bufs=2)
                            for j in range(2):
                                nc.tensor.matmul(
                                    st_ps[:, j, :],
                                    kTr[rr * 64:rr * 64 + 64,
                                        (2 * g + j) * 128:(2 * g + j + 1) * 128],
                                    q_rhs, start=True, stop=True,
                                )
                            if prev is not None:
                                av(prev)
                            e_sb = tmpB.tile([128, 2, 512], BF, tag="e")
                            nc.scalar.activation(e_sb[:], st_ps[:], AF.Exp, scale=SCALE)
                            prev = (g, e_sb)
                        av(prev)

                        recip = tmpB.tile([1, 512], F32, tag="recip")
                        nc.vector.reciprocal(recip[:], ot_ps[64:65, :])
                        bcast = tmpB.tile([64, 512], F32, tag="bcast")
                        nc.gpsimd.partition_broadcast(bcast[:], recip[:])
                        nc.vector.tensor_mul(
                            oT[jb][rr * 64:rr * 64 + 64, qsl], ot_ps[0:64, :], bcast[:]
                        )

                        # interleave stage C of the previous q span: one srow
                        # per head iteration -> PE filler work that keeps HAM
                        # warm during the exp-gated attention cadence
                        if qq > 0:
                            stage_c(qq - 1, h, h + 1)
                        if qq == NS - 1 and h == 1:
                            # heads 0,1 of the last span are done: run their
                            # half of the final output projection now so only
                            # the jb=1 half remains after the last AV
                            for st4 in range(4):
                                srow = qq * 4 + st4
                                for nn in range(NS):
                                    oc0 = psC.tile([128, 512], F32, tag="oc", bufs=2)
                                    nc.tensor.matmul(
                                        oc0[:], oT[0][:, srow * 128:(srow + 1) * 128],
                                        wo_sb[:, 0, bass.ts(nn, 512)], start=True, stop=True,
                                    )
                                    nc.vector.tensor_copy(ob0[:, st4 * 4 + nn, :], oc0[:])
                    if qq == NS - 1:
                        # tail: only the jb=1 half of the last span's stage C
                        for st4 in range(4):
                            srow = qq * 4 + st4
                            for nn in range(NS):
                                oc1 = psC.tile([128, 512], F32, tag="oc", bufs=2)
                                nc.tensor.matmul(
                                    oc1[:], oT[1][:, srow * 128:(srow + 1) * 128],
                                    wo_sb[:, 1, bass.ts(nn, 512)], start=True, stop=True,
                                )
                                ob = outp.tile([128, 512], F32, tag="ob")
                                nc.vector.tensor_add(ob[:], oc1[:], ob0[:, st4 * 4 + nn, :])
                                nc.sync.dma_start(
                                    out_d[srow * 128:(srow + 1) * 128, bass.ts(nn, 512)], ob[:]
                                )
    nc.compile()
    return nc


_NC_CACHE = None


def _get_program():
    global _NC_CACHE
    if _NC_CACHE is None:
        _NC_CACHE = _build_program()
    return _NC_CACHE


def _host_constants():
    inv_freq = 1.0 / (ROPE_BASE ** (np.arange(0, HD, 2, dtype=np.float32) / HD))
    t = np.arange(S, dtype=np.float32)
    freqs = np.outer(t, inv_freq)
    emb = np.concatenate([freqs, freqs], -1)          # [s, 64]
    cosT = np.cos(emb).T.astype(np.float32)           # [64, s]
    sinT = np.sin(emb).T.astype(np.float32)
    cos2 = np.ascontiguousarray(np.concatenate([cosT, cosT], 0))  # [128, s]
    sin2 = np.ascontiguousarray(np.concatenate([sinT, sinT], 0))

    R = np.zeros((HD, HD), np.float32)
    for i in range(32):
        R[i, i + 32] = -1.0
        R[i + 32, i] = 1.0
    RT = R.T
    rot_q = np.zeros((128, 128), np.float32)
    rot_q[0:64, 0:64] = RT
    rot_q[64:128, 64:128] = RT
    rot_k = np.zeros((128, 64), np.float32)
    rot_k[0:64, 0:64] = RT
    id64 = np.zeros((128, 64), np.float32)
    id64[64:128, :] = np.eye(64, dtype=np.float32)
    ones_col = np.ones((128, KC), np.float32)
    return cos2, sin2, rot_q, rot_k, id64, ones_col


def _bf(a):
    return np.ascontiguousarray(np.asarray(a, dtype=np.float32)).astype(NPBF)


def _in_maps(x, wq, wk, wv, wo):
    xT = _bf(x.reshape(S, D).T)
    cos2, sin2, rot_q, rot_k, id64, ones_col = _host_constants()
    cos2, sin2, rot_q, rot_k, id64, ones_col = (
        _bf(cos2), _bf(sin2), _bf(rot_q), _bf(rot_k), _bf(id64), _bf(ones_col)
    )
    maps = []
    for c in range(NCORES):
        wq_c = _bf(wq[:, c * QW:(c + 1) * QW])
        wkv_c = _bf(
            np.concatenate([wk[:, c * HD:(c + 1) * HD], wv[:, c * HD:(c + 1) * HD]], 1)
        )
        wo_c = _bf(wo[c * QW:(c + 1) * QW, :])
        maps.append({
            "xt": xT, "wq_s": wq_c, "wkv_s": wkv_c, "wo_s": wo_c,
            "cos2": cos2, "sin2": sin2, "rot_q": rot_q, "rot_k": rot_k,
            "id64": id64, "ones_col": ones_col,
        })
    return maps


def _run(in_maps, trace=False):
    nc = _get_program()
    return run_bass_kernel_spmd(nc, in_maps, core_ids=list(range(NCORES)), trace=trace)


def kernel(x, wq, wk, wv, wo):
    x, wq, wk, wv, wo = (np.asarray(a, dtype=np.float32) for a in (x, wq, wk, wv, wo))
    res = _run(_in_maps(x, wq, wk, wv, wo), trace=False)
    acc = res.results[0]["out"].astype(np.float64)
    for c in range(1, NCORES):
        acc += res.results[c]["out"]
    return acc.astype(np.float32).reshape(1, S, D)


def run_traced(x, wq, wk, wv, wo):
    """Like kernel() but with NTFF profiling; returns (out, BassKernelResults)."""
    x, wq, wk, wv, wo = (np.asarray(a, dtype=np.float32) for a in (x, wq, wk, wv, wo))
    res = _run(_in_maps(x, wq, wk, wv, wo), trace=True)
    acc = res.results[0]["out"].astype(np.float64)
    for c in range(1, NCORES):
        acc += res.results[c]["out"]
    return acc.astype(np.float32).reshape(1, S, D), res
